# revision 1
# baseline (speedup 1.0000x reference)
"""ChebConv (K=4) Trainium2 kernel.

Math (exactly matches the reference, which applies the spmm to `x` — not the
recurrence state — in every Chebyshev iteration):

    deg   = segment_sum(edge_weight, row)
    dinv  = deg^-1/2 (0 where deg <= 0)
    L[r,c]= sum over edges (r,c) of -2*dinv[r]*w*dinv[c];  L[i,i] += 2*fill
    Lx    = L @ x[b]                    (per batch)
    out   = x @ (W0 - W2) + Lx @ (W1 + 2*W2 + W3) + bias

Device strategy: densify L (pad N 10000->10240), shard its rows over 8 cores
(1280 rows each).  Per core the SpMM becomes a (1280 x 10240) @ (10240 x 512)
bf16 matmul (512 = 4 batches x 128 features concatenated) accumulated in fp32
PSUM, followed by fp32 feature-transform matmuls (x@A + Lx@B + bias) done in
transposed orientation so no device-side transpose of x is needed.
"""

import numpy as np
import ml_dtypes

B = 4
N_NODES = 10000
F = 128
SELF_LOOP_FILL = -0.05
NCORES = 8
NPAD = 10240                 # 80 tiles of 128; divisible by 8 cores
MROWS = NPAD // NCORES       # 1280 output rows per core
MT = MROWS // 128            # 10 m-tiles per core
KT = NPAD // 128             # 80 k-tiles
KCHUNK = 8                   # k-tiles per L/X DMA chunk
BF = B * F                   # 512 moving columns
# phase-1 m-groups: 6+4 m-tiles accumulate in parallel PSUM banks while k
# streams; group 0 is wide so its L-demand stays under the HBM rate while X
# streams alongside.
MGROUPS = [(0, 6), (6, 4)]           # (first m, count)
LM_OFFS = [0, KT * 6 * 128]          # free-dim offset of each group's L block
# k-chunk schedule: small leading chunks let the first matmuls start early
G0_CHUNKS = [2, 2, 4] + [8] * 9      # group 0 (also the X DMA schedule)
GN_CHUNKS = [8] * 10                 # later groups

_state = {}


def _build_nc():
    from contextlib import ExitStack

    import concourse.bass as bass
    import concourse.bacc as bacc
    import concourse.tile as tile
    from concourse import mybir

    dt = mybir.dt
    nc = bacc.Bacc(
        "TRN2", target_bir_lowering=False, debug=False, num_devices=NCORES
    )

    lmat = nc.declare_dram_parameter(
        "lmat", [128, KT * MROWS], dt.bfloat16, isOutput=False
    )
    xmat = nc.declare_dram_parameter("xmat", [128, KT, BF], dt.bfloat16, isOutput=False)
    xt = nc.declare_dram_parameter("xt", [128, B, MROWS], dt.float32, isOutput=False)
    wa = nc.declare_dram_parameter("wa", [128, 128], dt.float32, isOutput=False)
    wb = nc.declare_dram_parameter("wb", [128, 128], dt.float32, isOutput=False)
    biasv = nc.declare_dram_parameter("biasv", [128, 1], dt.float32, isOutput=False)
    ident = nc.declare_dram_parameter("ident", [128, 128], dt.float32, isOutput=False)
    out_t = nc.declare_dram_parameter("out_t", [B, 128, MROWS], dt.float32, isOutput=True)

    with ExitStack() as ctx:
        tc = ctx.enter_context(tile.TileContext(nc))
        const = ctx.enter_context(tc.tile_pool(name="const", bufs=1))
        xpool = ctx.enter_context(tc.tile_pool(name="xmat", bufs=1))
        lpool = ctx.enter_context(tc.tile_pool(name="lchunk", bufs=3))
        lxpool = ctx.enter_context(tc.tile_pool(name="lx", bufs=MT))
        lxtpool = ctx.enter_context(tc.tile_pool(name="lxt", bufs=1))
        outpool = ctx.enter_context(tc.tile_pool(name="outstg", bufs=3))
        # one shared PSUM pool: 6 phase-1 accumulators + warmup/transpose/
        # phase-2 tiles all rotate through the 8 banks
        psum = ctx.enter_context(
            tc.tile_pool(name="psum", bufs=8, space=bass.MemorySpace.PSUM)
        )
        psum1 = psumT = psum2 = psum

        # constants + xt on the scalar HWDGE queue (off the streaming path);
        # ident goes first — the PE warmup depends on it
        id_sb = const.tile([128, 128], dt.float32, tag="ident")
        nc.scalar.dma_start(id_sb[:], ident[:])
        wa_sb = const.tile([128, 128], dt.float32, tag="wa")
        nc.scalar.dma_start(wa_sb[:], wa[:])
        wb_sb = const.tile([128, 128], dt.float32, tag="wb")
        nc.scalar.dma_start(wb_sb[:], wb[:])
        bias_sb = const.tile([128, 1], dt.float32, tag="bias")
        nc.scalar.dma_start(bias_sb[:], biasv[:])
        xt_sb = const.tile([128, B, MROWS], dt.float32, tag="xt")

        xm_sb = xpool.tile([128, KT, BF], dt.bfloat16)
        lxT_sb = lxtpool.tile([128, B, MROWS], dt.float32)
        lx_tiles = [None] * MT

        # PE warmup: dummy matmuls on the identity as soon as it lands, so
        # the HAM clock-gate opens before the first real chunk arrives.
        pw = psum.tile([128, 128], dt.float32, tag="ps", name="ps_warm")
        for i in range(36):
            nc.tensor.matmul(
                pw[:], id_sb[:], id_sb[:], start=(i == 0), stop=(i == 35)
            )

        # Phase 1 (k-major): for each m-group, stream k-chunks of L (and, in
        # group 0, X) and accumulate the group's m-tiles in parallel PSUM
        # banks.
        for gi, (m0, gw) in enumerate(MGROUPS):
            wg = gw * 128
            chunks = G0_CHUNKS if gi == 0 else GN_CHUNKS
            pss = [
                psum.tile([128, BF], dt.float32, tag="ps", name=f"ps1_{m0 + i}")
                for i in range(gw)
            ]
            kt = 0
            off = LM_OFFS[gi]
            for kc in chunks:
                if gi == 0:
                    # X rides the scalar HWDGE queue so its prefetch is not
                    # gated by L's tile-slot flow control on the sync queue
                    nc.scalar.dma_start(
                        xm_sb[:, kt : kt + kc, :], xmat[:, kt : kt + kc, :]
                    )
                lt = lpool.tile([128, kc * wg], dt.bfloat16, tag="lt")
                nc.sync.dma_start(lt[:], lmat[:, off : off + kc * wg])
                off += kc * wg
                for j in range(kc):
                    for i in range(gw):
                        nc.tensor.matmul(
                            pss[i][:],
                            lt[:, j * wg + i * 128 : j * wg + (i + 1) * 128],
                            xm_sb[:, kt, :],
                            start=(kt == 0),
                            stop=(kt == KT - 1),
                        )
                    kt += 1
            for i in range(gw):
                lx_sb = lxpool.tile([128, BF], dt.float32, tag="lx")
                nc.vector.tensor_copy(lx_sb[:], pss[i][:])
                lx_tiles[m0 + i] = lx_sb

        # xt is only needed by phase 2 — load it while group 1 streams
        nc.scalar.dma_start(xt_sb[:], xt[:])

        # Phase 1.5: transpose Lx tiles (node-major -> feature-major)
        for m in range(MT):
            for b in range(B):
                pt = psumT.tile([128, 128], dt.float32, tag="ps", name=f"pt_{m}_{b}")
                nc.tensor.transpose(
                    pt[:], lx_tiles[m][:, b * 128 : (b + 1) * 128], id_sb[:]
                )
                nc.vector.tensor_copy(lxT_sb[:, b, m * 128 : (m + 1) * 128], pt[:])

        # Phase 2: out_T = A^T x^T + B^T Lx^T + bias  (fp32)
        starts = list(range(0, MROWS, 512))
        for b in range(B):
            for st in starts:
                csz = min(512, MROWS - st)
                ps2 = psum2.tile([128, 512], dt.float32, tag="ps", name=f"ps2_{b}_{st}")
                nc.tensor.matmul(
                    ps2[:, :csz], wa_sb[:], xt_sb[:, b, st : st + csz],
                    start=True, stop=False,
                )
                nc.tensor.matmul(
                    ps2[:, :csz], wb_sb[:], lxT_sb[:, b, st : st + csz],
                    start=False, stop=True,
                )
                ot = outpool.tile([128, 512], dt.float32, tag="ot")
                nc.scalar.activation(
                    ot[:, :csz], ps2[:, :csz],
                    mybir.ActivationFunctionType.Identity,
                    bias=bias_sb[:],
                )
                nc.scalar.dma_start(out_t[b, :, st : st + csz], ot[:, :csz])

    return nc


def _get_nc():
    if "nc" not in _state:
        nc = _build_nc()
        nc.compile()
        _state["nc"] = nc
    return _state["nc"]


def _prep_inputs(x, edge_index, edge_weight, weight, bias):
    """Host-side graph preprocessing -> per-core device input maps."""
    bf16 = ml_dtypes.bfloat16
    row = np.asarray(edge_index[0], dtype=np.int64)
    col = np.asarray(edge_index[1], dtype=np.int64)
    w = np.asarray(edge_weight, dtype=np.float32)

    deg = np.bincount(row, weights=w.astype(np.float64), minlength=N_NODES)
    deg = deg.astype(np.float32)
    dinv = np.where(deg > 0, np.where(deg > 0, deg, 1.0) ** -0.5, 0.0).astype(
        np.float32
    )
    lap2 = (-2.0 * dinv[row] * w * dinv[col]).astype(np.float32)

    # Dense transposed Laplacian: LT[src, dst] (lhsT orientation for the PE)
    LT = np.zeros((NPAD, NPAD), dtype=np.float32)
    np.add.at(LT, (col, row), lap2)
    idx = np.arange(N_NODES)
    LT[idx, idx] += 2.0 * SELF_LOOP_FILL
    LT16 = LT.astype(bf16)
    del LT

    # X in (node, batch*feat) layout, zero-padded rows
    xn = np.ascontiguousarray(np.transpose(x, (1, 0, 2)).reshape(N_NODES, BF))
    xn_pad = np.zeros((NPAD, BF), dtype=np.float32)
    xn_pad[:N_NODES] = xn
    # moving operand: (kr, kt, bf), node = kt*128 + kr
    xmat = np.ascontiguousarray(
        xn_pad.reshape(KT, 128, BF).transpose(1, 0, 2)
    ).astype(bf16)

    W = np.asarray(weight, dtype=np.float32)
    A = W[0] - W[2]
    Bm = W[1] + 2.0 * W[2] + W[3]
    biasv = np.asarray(bias, dtype=np.float32).reshape(128, 1)
    identity = np.eye(128, dtype=np.float32)

    in_maps = []
    for c in range(NCORES):
        r0, r1 = c * MROWS, (c + 1) * MROWS
        # per m-group block: [kr, kt, dst-in-group], groups concatenated on
        # the free dim; lmat[kr, off_g + (kt*gw*128 + dg)] = LT[kt*128+kr, ...]
        shard = LT16[:, r0:r1].reshape(KT, 128, MROWS)  # [kt, kr, dst]
        blocks = []
        for (m0, gw) in MGROUPS:
            blk = shard[:, :, m0 * 128 : (m0 + gw) * 128]  # (KT,128,gw*128)
            blocks.append(blk.transpose(1, 0, 2).reshape(128, KT * gw * 128))
        lmat = np.ascontiguousarray(np.concatenate(blocks, axis=1))
        # xt[f, b, nn] = x[b, r0+nn, f]
        xt = np.ascontiguousarray(xn_pad[r0:r1].reshape(MROWS, B, F).transpose(2, 1, 0))
        in_maps.append(
            {
                "lmat": lmat,
                "xmat": xmat,
                "xt": xt,
                "wa": A,
                "wb": Bm,
                "biasv": biasv,
                "ident": identity,
            }
        )
    return in_maps


def _ensure_ntff_hook():
    """Register the axon NTFF profiling hook if the image's antenv lacks it.

    The boot path degrades silently when ``antenv.axon_hooks`` is missing;
    recreate the tiny get/set holder and wire it to libaxon_pjrt.so so
    ``run_bass_kernel_spmd(trace=True)`` can capture NTFF profiles.
    """
    import sys
    import types

    try:
        from antenv.axon_hooks import get_axon_ntff_profile_hook  # noqa: F401

        return
    except ImportError:
        pass
    mod = types.ModuleType("antenv.axon_hooks")
    holder = {}
    mod.set_axon_ntff_profile_hook = lambda h: holder.__setitem__("h", h)
    mod.get_axon_ntff_profile_hook = lambda: holder.get("h")
    sys.modules["antenv.axon_hooks"] = mod
    import antenv

    antenv.axon_hooks = mod
    from trn_agent_boot.trn_boot import _ntff_profile_via_ctypes

    hook = _ntff_profile_via_ctypes("/opt/axon/libaxon_pjrt.so")
    if hook is not None:
        mod.set_axon_ntff_profile_hook(hook)


def kernel(x, edge_index, edge_weight, weight, bias):
    import os

    from concourse.bass_utils import run_bass_kernel_spmd

    x = np.asarray(x, dtype=np.float32)
    in_maps = _prep_inputs(x, edge_index, edge_weight, weight, bias)
    nc = _get_nc()
    trace = bool(int(os.environ.get("CHEB_TRACE", "0")))
    if trace:
        _ensure_ntff_hook()
    res = run_bass_kernel_spmd(nc, in_maps, list(range(NCORES)), trace=trace)
    _state["last_result"] = res
    out_T = np.concatenate([res.results[c]["out_t"] for c in range(NCORES)], axis=2)
    out = np.ascontiguousarray(out_T.transpose(0, 2, 1)[:, :N_NODES, :])
    return out



# revision 2
# speedup vs baseline: 1.6556x; 1.6556x over previous
"""ChebConv (K=4) Trainium2 kernel — sparse scatter-matmul version.

Math (exactly matches the reference, which applies the spmm to `x` — not the
recurrence state — in every Chebyshev iteration):

    deg   = segment_sum(edge_weight, row)
    dinv  = deg^-1/2 (0 where deg <= 0)
    L[r,c]= sum over edges (r,c) of -2*dinv[r]*w*dinv[c];  L[i,i] += 2*fill
    Lx    = L @ x[b]                    (per batch)
    out   = x @ (W0 - W2) + Lx @ (W1 + 2*W2 + W3) + bias

Device strategy: the graph is 0.16% dense (160k edges + 10k self loops over
10000^2), so instead of densifying L (80x80 grid of 128x128 tiles, 800
matmuls/core), exploit sparsity.  Host buckets edges by destination row into
8 cores x 10 windows of 128 rows, padding each bucket to whole chunks of 128
edges.  For chunk ci of window w the device computes

    Lx[w] += P_ci^T @ Xg_ci

where P_ci[e, j] = lap(e) if edge e's dst row (within the window) == j else 0
(stationary operand, built host-side with the lap value folded in), and
Xg_ci[e, :] = x[:, src(e), :] (host-gathered source rows; 512 cols = 4
batches x 128 feats, bf16).  ~18 chunks/window -> ~180 matmuls/core instead
of 800, and the PE streams 24 MB of gathered rows + 6 MB of P instead of a
26 MB dense Laplacian + 10.5 MB X.  Fp32 feature-transform matmuls
(x@A + Lx@B + bias) are done in transposed orientation as before.
"""

import numpy as np
import ml_dtypes

B = 4
N_NODES = 10000
F = 128
SELF_LOOP_FILL = -0.05
NCORES = 8
NPAD = 10240                 # 80 tiles of 128; divisible by 8 cores
MROWS = NPAD // NCORES       # 1280 output rows per core
MT = MROWS // 128            # 10 dst-row windows per core
BF = B * F                   # 512 moving columns

_state = {}


def _build_nc(cpw):
    from contextlib import ExitStack

    import concourse.bass as bass
    import concourse.bacc as bacc
    import concourse.tile as tile
    from concourse import mybir

    dt = mybir.dt
    nc = bacc.Bacc(
        "TRN2", target_bir_lowering=False, debug=False, num_devices=NCORES
    )

    ct = int(sum(cpw))
    pmat = nc.declare_dram_parameter(
        "pmat", [128, ct * 128], dt.bfloat16, isOutput=False
    )
    xg = nc.declare_dram_parameter("xg", [128, ct * BF], dt.bfloat16, isOutput=False)
    xt = nc.declare_dram_parameter("xt", [128, B, MROWS], dt.float32, isOutput=False)
    wa = nc.declare_dram_parameter("wa", [128, 128], dt.float32, isOutput=False)
    wb = nc.declare_dram_parameter("wb", [128, 128], dt.float32, isOutput=False)
    biasv = nc.declare_dram_parameter("biasv", [128, 1], dt.float32, isOutput=False)
    ident = nc.declare_dram_parameter("ident", [128, 128], dt.float32, isOutput=False)
    out_t = nc.declare_dram_parameter("out_t", [B, 128, MROWS], dt.float32, isOutput=True)

    with ExitStack() as ctx:
        tc = ctx.enter_context(tile.TileContext(nc))
        const = ctx.enter_context(tc.tile_pool(name="const", bufs=1))
        ppool = ctx.enter_context(tc.tile_pool(name="pchunk", bufs=3))
        xgpool = ctx.enter_context(tc.tile_pool(name="xgchunk", bufs=3))
        lxpool = ctx.enter_context(tc.tile_pool(name="lx", bufs=MT))
        lxtpool = ctx.enter_context(tc.tile_pool(name="lxt", bufs=1))
        outpool = ctx.enter_context(tc.tile_pool(name="outstg", bufs=3))
        # one shared PSUM pool: window accumulators + warmup/transpose/
        # phase-2 tiles all rotate through the 8 banks
        psum = ctx.enter_context(
            tc.tile_pool(name="psum", bufs=8, space=bass.MemorySpace.PSUM)
        )

        # constants + xt on the scalar HWDGE queue (off the streaming path);
        # ident goes first — the PE warmup depends on it
        id_sb = const.tile([128, 128], dt.float32, tag="ident")
        nc.scalar.dma_start(id_sb[:], ident[:])
        wa_sb = const.tile([128, 128], dt.float32, tag="wa")
        nc.scalar.dma_start(wa_sb[:], wa[:])
        wb_sb = const.tile([128, 128], dt.float32, tag="wb")
        nc.scalar.dma_start(wb_sb[:], wb[:])
        bias_sb = const.tile([128, 1], dt.float32, tag="bias")
        nc.scalar.dma_start(bias_sb[:], biasv[:])
        xt_sb = const.tile([128, B, MROWS], dt.float32, tag="xt")

        lxT_sb = lxtpool.tile([128, B, MROWS], dt.float32)
        lx_tiles = [None] * MT

        # PE warmup: dummy matmuls on the identity as soon as it lands, so
        # the HAM clock-gate opens before the first real chunk arrives.
        pw = psum.tile([128, 128], dt.float32, tag="ps", name="ps_warm")
        for i in range(36):
            nc.tensor.matmul(
                pw[:], id_sb[:], id_sb[:], start=(i == 0), stop=(i == 35)
            )

        # Phase 1 (sparse scatter): per dst window, stream the P chunks and
        # the gathered source rows, accumulating the window in one PSUM bank.
        offp = 0
        offx = 0
        for w in range(MT):
            cp = cpw[w]
            pt = ppool.tile([128, cp * 128], dt.bfloat16, tag="pt")
            nc.sync.dma_start(pt[:], pmat[:, offp : offp + cp * 128])
            xgt = xgpool.tile([128, cp * BF], dt.bfloat16, tag="xgt")
            nc.sync.dma_start(xgt[:], xg[:, offx : offx + cp * BF])
            offp += cp * 128
            offx += cp * BF
            ps = psum.tile([128, BF], dt.float32, tag="ps", name=f"ps1_{w}")
            for ci in range(cp):
                nc.tensor.matmul(
                    ps[:],
                    pt[:, ci * 128 : (ci + 1) * 128],
                    xgt[:, ci * BF : (ci + 1) * BF],
                    start=(ci == 0),
                    stop=(ci == cp - 1),
                )
            lx_sb = lxpool.tile([128, BF], dt.float32, tag="lx")
            nc.vector.tensor_copy(lx_sb[:], ps[:])
            lx_tiles[w] = lx_sb
            # xt is only needed by phase 2 — load it mid-stream
            if w == 5:
                nc.scalar.dma_start(xt_sb[:], xt[:])

        # Phase 1.5: transpose Lx tiles (node-major -> feature-major)
        for m in range(MT):
            for b in range(B):
                pt2 = psum.tile([128, 128], dt.float32, tag="ps", name=f"pt_{m}_{b}")
                nc.tensor.transpose(
                    pt2[:], lx_tiles[m][:, b * 128 : (b + 1) * 128], id_sb[:]
                )
                nc.vector.tensor_copy(lxT_sb[:, b, m * 128 : (m + 1) * 128], pt2[:])

        # Phase 2: out_T = A^T x^T + B^T Lx^T + bias  (fp32)
        starts = list(range(0, MROWS, 512))
        for b in range(B):
            for st in starts:
                csz = min(512, MROWS - st)
                ps2 = psum.tile([128, 512], dt.float32, tag="ps", name=f"ps2_{b}_{st}")
                nc.tensor.matmul(
                    ps2[:, :csz], wa_sb[:], xt_sb[:, b, st : st + csz],
                    start=True, stop=False,
                )
                nc.tensor.matmul(
                    ps2[:, :csz], wb_sb[:], lxT_sb[:, b, st : st + csz],
                    start=False, stop=True,
                )
                ot = outpool.tile([128, 512], dt.float32, tag="ot")
                nc.scalar.activation(
                    ot[:, :csz], ps2[:, :csz],
                    mybir.ActivationFunctionType.Identity,
                    bias=bias_sb[:],
                )
                nc.scalar.dma_start(out_t[b, :, st : st + csz], ot[:, :csz])

    return nc


def _get_nc(cpw):
    key = ("nc", tuple(cpw))
    if key not in _state:
        nc = _build_nc(cpw)
        nc.compile()
        _state[key] = nc
    return _state[key]


def _prep_inputs(x, edge_index, edge_weight, weight, bias):
    """Host-side graph preprocessing -> per-core device input maps."""
    bf16 = ml_dtypes.bfloat16
    row = np.asarray(edge_index[0], dtype=np.int64)
    col = np.asarray(edge_index[1], dtype=np.int64)
    w = np.asarray(edge_weight, dtype=np.float32)

    deg = np.bincount(row, weights=w.astype(np.float64), minlength=N_NODES)
    deg = deg.astype(np.float32)
    dinv = np.where(deg > 0, np.where(deg > 0, deg, 1.0) ** -0.5, 0.0).astype(
        np.float32
    )
    lap2 = (-2.0 * dinv[row] * w * dinv[col]).astype(np.float32)

    # append self loops as ordinary edges
    loops = np.arange(N_NODES, dtype=np.int64)
    rows_all = np.concatenate([row, loops])
    cols_all = np.concatenate([col, loops])
    laps_all = np.concatenate(
        [lap2, np.full(N_NODES, 2.0 * SELF_LOOP_FILL, np.float32)]
    )

    # bucket edges by (core, window) = destination row // 128, derive a
    # shared (SPMD) chunks-per-window schedule covering the fullest core
    g = rows_all // 128                                  # global window id
    cnt = np.bincount(g, minlength=NCORES * MT)
    cpw = np.maximum(
        (cnt.reshape(NCORES, MT).max(axis=0) + 127) // 128, 1
    ).astype(np.int64)
    ct = int(cpw.sum())                                  # chunks per core
    cum = np.zeros(MT, np.int64)
    cum[1:] = np.cumsum(cpw)[:-1]

    order = np.argsort(g, kind="stable")
    gs = g[order]
    starts = np.zeros(NCORES * MT + 1, np.int64)
    starts[1:] = np.cumsum(cnt)
    rank = np.arange(gs.size, dtype=np.int64) - starts[gs]
    cs = gs // MT
    slot = cum[gs % MT] * 128 + rank                     # slot within core
    p = slot % 128                                       # partition (edge lane)
    ci = slot // 128                                     # chunk within core
    j = rows_all[order] % 128                            # dst row within window

    srcs = np.zeros((NCORES, ct * 128), np.int64)
    srcs[cs, slot] = cols_all[order]
    pm = np.zeros((NCORES, 128, ct * 128), np.float32)
    pm[cs, p, ci * 128 + j] = laps_all[order]
    pmat = pm.astype(bf16)
    del pm

    xf = np.asarray(x, np.float32)
    W = np.asarray(weight, dtype=np.float32)
    A = W[0] - W[2]
    Bm = W[1] + 2.0 * W[2] + W[3]
    biasv = np.asarray(bias, dtype=np.float32).reshape(128, 1)
    identity = np.eye(128, dtype=np.float32)

    xn_pad = np.zeros((NPAD, B, F), np.float32)
    xn_pad[:N_NODES] = np.transpose(xf, (1, 0, 2))

    in_maps = []
    for c in range(NCORES):
        S = srcs[c].reshape(ct, 128)
        # xg[e_lane, ci, b*128+f] = x[b, src(ci, e_lane), f]
        xgc = np.ascontiguousarray(
            np.transpose(xf[:, S, :], (2, 1, 0, 3)).reshape(128, ct * BF)
        ).astype(bf16)
        r0 = c * MROWS
        # xt[f, b, nn] = x[b, r0+nn, f]
        xtc = np.ascontiguousarray(xn_pad[r0 : r0 + MROWS].transpose(2, 1, 0))
        in_maps.append(
            {
                "pmat": np.ascontiguousarray(pmat[c]),
                "xg": xgc,
                "xt": xtc,
                "wa": A,
                "wb": Bm,
                "biasv": biasv,
                "ident": identity,
            }
        )
    return in_maps, tuple(int(v) for v in cpw)


def _ensure_ntff_hook():
    """Register the axon NTFF profiling hook if the image's antenv lacks it.

    The boot path degrades silently when ``antenv.axon_hooks`` is missing;
    recreate the tiny get/set holder and wire it to libaxon_pjrt.so so
    ``run_bass_kernel_spmd(trace=True)`` can capture NTFF profiles.
    """
    import sys
    import types

    try:
        from antenv.axon_hooks import get_axon_ntff_profile_hook  # noqa: F401

        return
    except ImportError:
        pass
    mod = types.ModuleType("antenv.axon_hooks")
    holder = {}
    mod.set_axon_ntff_profile_hook = lambda h: holder.__setitem__("h", h)
    mod.get_axon_ntff_profile_hook = lambda: holder.get("h")
    sys.modules["antenv.axon_hooks"] = mod
    import antenv

    antenv.axon_hooks = mod
    from trn_agent_boot.trn_boot import _ntff_profile_via_ctypes

    hook = _ntff_profile_via_ctypes("/opt/axon/libaxon_pjrt.so")
    if hook is not None:
        mod.set_axon_ntff_profile_hook(hook)


def kernel(x, edge_index, edge_weight, weight, bias):
    import os

    from concourse.bass_utils import run_bass_kernel_spmd

    x = np.asarray(x, dtype=np.float32)
    in_maps, cpw = _prep_inputs(x, edge_index, edge_weight, weight, bias)
    nc = _get_nc(cpw)
    trace = bool(int(os.environ.get("CHEB_TRACE", "0")))
    if trace:
        _ensure_ntff_hook()
    res = run_bass_kernel_spmd(nc, in_maps, list(range(NCORES)), trace=trace)
    _state["last_result"] = res
    out_T = np.concatenate([res.results[c]["out_t"] for c in range(NCORES)], axis=2)
    out = np.ascontiguousarray(out_T.transpose(0, 2, 1)[:, :N_NODES, :])
    return out


# revision 3
# speedup vs baseline: 1.9125x; 1.1552x over previous
"""ChebConv (K=4) Trainium2 kernel — sparse scatter-matmul version.

Math (exactly matches the reference, which applies the spmm to `x` — not the
recurrence state — in every Chebyshev iteration):

    deg   = segment_sum(edge_weight, row)
    dinv  = deg^-1/2 (0 where deg <= 0)
    L[r,c]= sum over edges (r,c) of -2*dinv[r]*w*dinv[c];  L[i,i] += 2*fill
    Lx    = L @ x[b]                    (per batch)
    out   = x @ (W0 - W2) + Lx @ (W1 + 2*W2 + W3) + bias

Device strategy: the graph is 0.16% dense (160k edges + 10k self loops over
10000^2), so instead of densifying L, exploit sparsity.  Host buckets edges
by destination row into 8 cores x 10 windows of 128 rows, padding each
bucket to whole chunks of 128 edges.  For chunk ci of window w the device
computes

    Lx[w] += P_ci^T @ Xg_ci

where P_ci[e, j] = lap(e) if edge e's dst row (within the window) == j
else 0 (stationary operand, lap folded in host-side), and Xg_ci[e, :] =
x[:, src(e), :] (host-gathered source rows; 512 cols = 4 batches x 128
feats).  ~18 chunks/window -> ~180 matmuls/core instead of the dense 800,
streaming 24 MB of gathered rows + 6 MB of P per core.

The epilogue (transpose Lx to feature-major, apply the two weight matmuls,
add bias, DMA out) runs per window, lagged one window behind the scatter,
in bf16 — so the whole kernel is a single DMA-paced pipeline with no
serial tail.
"""

import numpy as np
import ml_dtypes

B = 4
N_NODES = 10000
F = 128
SELF_LOOP_FILL = -0.05
NCORES = 8
NPAD = 10240                 # 80 tiles of 128; divisible by 8 cores
MROWS = NPAD // NCORES       # 1280 output rows per core
MT = MROWS // 128            # 10 dst-row windows per core
BF = B * F                   # 512 moving columns

_state = {}


def _build_nc(cpw):
    from contextlib import ExitStack

    import concourse.bass as bass
    import concourse.bacc as bacc
    import concourse.tile as tile
    from concourse import mybir

    dt = mybir.dt
    nc = bacc.Bacc(
        "TRN2", target_bir_lowering=False, debug=False, num_devices=NCORES
    )

    ct = int(sum(cpw))
    pmat = nc.declare_dram_parameter(
        "pmat", [128, ct * 128], dt.bfloat16, isOutput=False
    )
    xg = nc.declare_dram_parameter("xg", [128, ct * BF], dt.bfloat16, isOutput=False)
    xt = nc.declare_dram_parameter("xt", [128, MT, BF], dt.bfloat16, isOutput=False)
    wa = nc.declare_dram_parameter("wa", [128, 128], dt.bfloat16, isOutput=False)
    wb = nc.declare_dram_parameter("wb", [128, 128], dt.bfloat16, isOutput=False)
    biasv = nc.declare_dram_parameter("biasv", [128, 1], dt.float32, isOutput=False)
    ident = nc.declare_dram_parameter("ident", [128, 128], dt.bfloat16, isOutput=False)
    out_t = nc.declare_dram_parameter(
        "out_t", [MT, 128, BF], dt.bfloat16, isOutput=True
    )

    with ExitStack() as ctx:
        tc = ctx.enter_context(tile.TileContext(nc))
        const = ctx.enter_context(tc.tile_pool(name="const", bufs=1))
        ppool = ctx.enter_context(tc.tile_pool(name="pchunk", bufs=3))
        xgpool = ctx.enter_context(tc.tile_pool(name="xgchunk", bufs=3))
        lxnpool = ctx.enter_context(tc.tile_pool(name="lxn", bufs=2))
        lxtpool = ctx.enter_context(tc.tile_pool(name="lxt", bufs=2))
        outpool = ctx.enter_context(tc.tile_pool(name="outstg", bufs=2))
        psA = ctx.enter_context(
            tc.tile_pool(name="psA", bufs=2, space=bass.MemorySpace.PSUM)
        )
        psT = ctx.enter_context(
            tc.tile_pool(name="psT", bufs=2, space=bass.MemorySpace.PSUM)
        )
        psB = ctx.enter_context(
            tc.tile_pool(name="psB", bufs=2, space=bass.MemorySpace.PSUM)
        )

        # constants + xt on the scalar HWDGE queue (off the streaming path)
        id_sb = const.tile([128, 128], dt.bfloat16, tag="ident")
        nc.scalar.dma_start(id_sb[:], ident[:])
        wa_sb = const.tile([128, 128], dt.bfloat16, tag="wa")
        nc.scalar.dma_start(wa_sb[:], wa[:])
        wb_sb = const.tile([128, 128], dt.bfloat16, tag="wb")
        nc.scalar.dma_start(wb_sb[:], wb[:])
        bias_sb = const.tile([128, 1], dt.float32, tag="bias")
        nc.scalar.dma_start(bias_sb[:], biasv[:])
        xt_sb = const.tile([128, MT, BF], dt.bfloat16, tag="xt")
        nc.scalar.dma_start(xt_sb[:], xt[:])

        # PE warmup without any DMA dependency: memset a tile, then dummy
        # matmuls so the HAM clock-gate opens before the first real chunk.
        wz = const.tile([128, 128], dt.bfloat16, tag="wz")
        nc.vector.memset(wz[:], 0.0)
        pwarm = psA.tile([128, 128], dt.float32, tag="ps", name="ps_warm")
        for i in range(36):
            nc.tensor.matmul(
                pwarm[:], wz[:], wz[:], start=(i == 0), stop=(i == 35)
            )

        lxn_tiles = [None] * MT
        ps1_tiles = [None] * MT

        def scatter(w, offp, offx):
            cp = cpw[w]
            pt = ppool.tile([128, cp * 128], dt.bfloat16, tag="pt")
            nc.sync.dma_start(pt[:], pmat[:, offp : offp + cp * 128])
            xgt = xgpool.tile([128, cp * BF], dt.bfloat16, tag="xgt")
            nc.sync.dma_start(xgt[:], xg[:, offx : offx + cp * BF])
            ps1 = psA.tile([128, BF], dt.float32, tag="ps", name=f"ps1_{w}")
            for ci in range(cp):
                nc.tensor.matmul(
                    ps1[:],
                    pt[:, ci * 128 : (ci + 1) * 128],
                    xgt[:, ci * BF : (ci + 1) * BF],
                    start=(ci == 0),
                    stop=(ci == cp - 1),
                )
            ps1_tiles[w] = ps1

        def epilogue(w):
            # node-major Lx (psum fp32) -> sbuf bf16
            lxn = lxnpool.tile([128, BF], dt.bfloat16, tag="lxn")
            nc.vector.tensor_copy(lxn[:], ps1_tiles[w][:])
            # transpose per batch: [node, f] -> [f, node]
            ptr = psT.tile([128, BF], dt.bfloat16, tag="pst", name=f"pst_{w}")
            for b in range(B):
                nc.tensor.transpose(
                    ptr[:, b * 128 : (b + 1) * 128],
                    lxn[:, b * 128 : (b + 1) * 128],
                    id_sb[:],
                )
            lxt = lxtpool.tile([128, BF], dt.bfloat16, tag="lxt")
            nc.vector.tensor_copy(lxt[:], ptr[:])
            # out_T = A^T x^T + B^T Lx^T (+bias), one 128-col slab per batch
            ps2 = psB.tile([128, BF], dt.float32, tag="ps2", name=f"ps2_{w}")
            for b in range(B):
                sl = slice(b * 128, (b + 1) * 128)
                nc.tensor.matmul(
                    ps2[:, sl], wa_sb[:], xt_sb[:, w, sl], start=True, stop=False
                )
                nc.tensor.matmul(
                    ps2[:, sl], wb_sb[:], lxt[:, sl], start=False, stop=True
                )
            ot = outpool.tile([128, BF], dt.bfloat16, tag="ot")
            nc.scalar.activation(
                ot[:], ps2[:],
                mybir.ActivationFunctionType.Identity,
                bias=bias_sb[:],
            )
            nc.gpsimd.dma_start(out_t[w], ot[:])

        # software-pipelined: epilogue of window w is emitted after the
        # scatter of window w+1, so the PE never waits on the vector copies
        offp = offx = 0
        for w in range(MT):
            scatter(w, offp, offx)
            offp += cpw[w] * 128
            offx += cpw[w] * BF
            if w > 0:
                epilogue(w - 1)
        epilogue(MT - 1)

    return nc


def _get_nc(cpw):
    key = ("nc", tuple(cpw))
    if key not in _state:
        nc = _build_nc(cpw)
        nc.compile()
        _state[key] = nc
    return _state[key]


def _prep_inputs(x, edge_index, edge_weight, weight, bias):
    """Host-side graph preprocessing -> per-core device input maps."""
    bf16 = ml_dtypes.bfloat16
    row = np.asarray(edge_index[0], dtype=np.int64)
    col = np.asarray(edge_index[1], dtype=np.int64)
    w = np.asarray(edge_weight, dtype=np.float32)

    deg = np.bincount(row, weights=w.astype(np.float64), minlength=N_NODES)
    deg = deg.astype(np.float32)
    dinv = np.where(deg > 0, np.where(deg > 0, deg, 1.0) ** -0.5, 0.0).astype(
        np.float32
    )
    lap2 = (-2.0 * dinv[row] * w * dinv[col]).astype(np.float32)

    # append self loops as ordinary edges
    loops = np.arange(N_NODES, dtype=np.int64)
    rows_all = np.concatenate([row, loops])
    cols_all = np.concatenate([col, loops])
    laps_all = np.concatenate(
        [lap2, np.full(N_NODES, 2.0 * SELF_LOOP_FILL, np.float32)]
    )

    # bucket edges by (core, window) = destination row // 128, derive a
    # shared (SPMD) chunks-per-window schedule covering the fullest core
    g = rows_all // 128                                  # global window id
    cnt = np.bincount(g, minlength=NCORES * MT)
    cpw = np.maximum(
        (cnt.reshape(NCORES, MT).max(axis=0) + 127) // 128, 1
    ).astype(np.int64)
    ct = int(cpw.sum())                                  # chunks per core
    cum = np.zeros(MT, np.int64)
    cum[1:] = np.cumsum(cpw)[:-1]

    order = np.argsort(g, kind="stable")
    gs = g[order]
    starts = np.zeros(NCORES * MT + 1, np.int64)
    starts[1:] = np.cumsum(cnt)
    rank = np.arange(gs.size, dtype=np.int64) - starts[gs]
    cs = gs // MT
    slot = cum[gs % MT] * 128 + rank                     # slot within core
    p = slot % 128                                       # partition (edge lane)
    ci = slot // 128                                     # chunk within core
    j = rows_all[order] % 128                            # dst row within window

    srcs = np.zeros((NCORES, ct * 128), np.int64)
    srcs[cs, slot] = cols_all[order]
    pm = np.zeros((NCORES, 128, ct * 128), np.float32)
    pm[cs, p, ci * 128 + j] = laps_all[order]
    pmat = pm.astype(bf16)
    del pm

    xf = np.asarray(x, np.float32)
    W = np.asarray(weight, dtype=np.float32)
    A = W[0] - W[2]
    Bm = W[1] + 2.0 * W[2] + W[3]
    biasv = np.asarray(bias, dtype=np.float32).reshape(128, 1)
    identity = np.eye(128, dtype=np.float32)

    xn_pad = np.zeros((NPAD, B, F), np.float32)
    xn_pad[:N_NODES] = np.transpose(xf, (1, 0, 2))

    in_maps = []
    for c in range(NCORES):
        S = srcs[c].reshape(ct, 128)
        # xg[e_lane, ci, b*128+f] = x[b, src(ci, e_lane), f]
        xgc = np.ascontiguousarray(
            np.transpose(xf[:, S, :], (2, 1, 0, 3)).reshape(128, ct * BF)
        ).astype(bf16)
        r0 = c * MROWS
        # xt[f, w, b*128+j] = x[b, r0 + w*128 + j, f]
        xtc = np.ascontiguousarray(
            xn_pad[r0 : r0 + MROWS]
            .reshape(MT, 128, B, F)
            .transpose(3, 0, 2, 1)
            .reshape(128, MT, BF)
        ).astype(bf16)
        in_maps.append(
            {
                "pmat": np.ascontiguousarray(pmat[c]),
                "xg": xgc,
                "xt": xtc,
                "wa": A.astype(bf16),
                "wb": Bm.astype(bf16),
                "biasv": biasv,
                "ident": identity.astype(bf16),
            }
        )
    return in_maps, tuple(int(v) for v in cpw)


def _ensure_ntff_hook():
    """Register the axon NTFF profiling hook if the image's antenv lacks it.

    The boot path degrades silently when ``antenv.axon_hooks`` is missing;
    recreate the tiny get/set holder and wire it to libaxon_pjrt.so so
    ``run_bass_kernel_spmd(trace=True)`` can capture NTFF profiles.
    """
    import sys
    import types

    try:
        from antenv.axon_hooks import get_axon_ntff_profile_hook  # noqa: F401

        return
    except ImportError:
        pass
    mod = types.ModuleType("antenv.axon_hooks")
    holder = {}
    mod.set_axon_ntff_profile_hook = lambda h: holder.__setitem__("h", h)
    mod.get_axon_ntff_profile_hook = lambda: holder.get("h")
    sys.modules["antenv.axon_hooks"] = mod
    import antenv

    antenv.axon_hooks = mod
    from trn_agent_boot.trn_boot import _ntff_profile_via_ctypes

    hook = _ntff_profile_via_ctypes("/opt/axon/libaxon_pjrt.so")
    if hook is not None:
        mod.set_axon_ntff_profile_hook(hook)


def kernel(x, edge_index, edge_weight, weight, bias):
    import os

    from concourse.bass_utils import run_bass_kernel_spmd

    x = np.asarray(x, dtype=np.float32)
    in_maps, cpw = _prep_inputs(x, edge_index, edge_weight, weight, bias)
    nc = _get_nc(cpw)
    trace = bool(int(os.environ.get("CHEB_TRACE", "0")))
    if trace:
        _ensure_ntff_hook()
    res = run_bass_kernel_spmd(nc, in_maps, list(range(NCORES)), trace=trace)
    _state["last_result"] = res
    # out_t[w, f, b*128+j] (per core) -> out[b, c*MROWS + w*128 + j, f]
    parts = []
    for c in range(NCORES):
        r = np.asarray(res.results[c]["out_t"], dtype=np.float32)
        parts.append(
            r.reshape(MT, 128, B, 128).transpose(2, 0, 3, 1).reshape(B, MROWS, F)
        )
    out = np.concatenate(parts, axis=1)[:, :N_NODES, :]
    return np.ascontiguousarray(out)


# revision 9
# speedup vs baseline: 2.7600x; 1.4431x over previous
"""ChebConv (K=4) Trainium2 kernel — sparse scatter-matmul version.

Math (exactly matches the reference, which applies the spmm to `x` — not the
recurrence state — in every Chebyshev iteration):

    deg   = segment_sum(edge_weight, row)
    dinv  = deg^-1/2 (0 where deg <= 0)
    L[r,c]= sum over edges (r,c) of -2*dinv[r]*w*dinv[c];  L[i,i] += 2*fill
    Lx    = L @ x[b]                    (per batch)
    out   = x @ (W0 - W2) + Lx @ (W1 + 2*W2 + W3) + bias

Device strategy: the graph is 0.16% dense (160k edges + 10k self loops over
10000^2), so instead of densifying L, exploit sparsity.  Host buckets edges
by destination row into 8 cores x 10 windows of 128 rows, padding each
bucket to whole chunks of 128 edges.  For chunk ci of window w the device
computes

    Lx[w] += P_ci^T @ Xg_ci

where P_ci[e, j] = lap(e) if edge e's dst row (within the window) == j
else 0 (stationary operand, lap folded in host-side), and Xg_ci[e, :] =
x[:, src(e), :] (host-gathered source rows; 512 cols = 4 batches x 128
feats).  ~18 chunks/window -> ~180 matmuls/core instead of the dense 800,
streaming 24 MB of gathered rows + 6 MB of P per core.

The epilogue (transpose Lx to feature-major, apply the two weight matmuls,
add bias, DMA out) runs per window, lagged one window behind the scatter,
in bf16 — so the whole kernel is a single DMA-paced pipeline with no
serial tail.
"""

import numpy as np
import ml_dtypes

B = 4
N_NODES = 10000
F = 128
SELF_LOOP_FILL = -0.05
NCORES = 8
NPAD = 10240                 # 80 tiles of 128; divisible by 8 cores
MROWS = NPAD // NCORES       # 1280 output rows per core
MT = MROWS // 128            # 10 dst-row windows per core
BF = B * F                   # 512 moving columns

_state = {}


def _build_nc(cpw):
    from contextlib import ExitStack

    import concourse.bass as bass
    import concourse.bacc as bacc
    import concourse.tile as tile
    from concourse import mybir

    dt = mybir.dt
    nc = bacc.Bacc(
        "TRN2", target_bir_lowering=False, debug=False, num_devices=NCORES
    )

    ct = int(sum(cpw))
    pmat = nc.declare_dram_parameter(
        "pmat", [128, ct * 128], dt.bfloat16, isOutput=False
    )
    xg = nc.declare_dram_parameter("xg", [128, ct * BF], dt.float8e3, isOutput=False)
    xt = nc.declare_dram_parameter("xt", [128, MT, BF], dt.bfloat16, isOutput=False)
    wa = nc.declare_dram_parameter("wa", [128, 128], dt.bfloat16, isOutput=False)
    wb = nc.declare_dram_parameter("wb", [128, 128], dt.bfloat16, isOutput=False)
    biasv = nc.declare_dram_parameter("biasv", [128, 1], dt.float32, isOutput=False)
    ident = nc.declare_dram_parameter("ident", [128, 128], dt.bfloat16, isOutput=False)
    out_t = nc.declare_dram_parameter(
        "out_t", [MT, 128, BF], dt.bfloat16, isOutput=True
    )

    with ExitStack() as ctx:
        tc = ctx.enter_context(tile.TileContext(nc))
        const = ctx.enter_context(tc.tile_pool(name="const", bufs=1))
        ppool = ctx.enter_context(tc.tile_pool(name="pchunk", bufs=4))
        xgpool = ctx.enter_context(tc.tile_pool(name="xgchunk", bufs=4))
        lxnpool = ctx.enter_context(tc.tile_pool(name="lxn", bufs=2))
        lxtpool = ctx.enter_context(tc.tile_pool(name="lxt", bufs=3))
        outpool = ctx.enter_context(tc.tile_pool(name="outstg", bufs=2))
        psA = ctx.enter_context(
            tc.tile_pool(name="psA", bufs=2, space=bass.MemorySpace.PSUM)
        )
        psT = ctx.enter_context(
            tc.tile_pool(name="psT", bufs=2, space=bass.MemorySpace.PSUM)
        )
        psB = ctx.enter_context(
            tc.tile_pool(name="psB", bufs=2, space=bass.MemorySpace.PSUM)
        )

        # constants + xt on the scalar HWDGE queue (off the streaming path)
        id_sb = const.tile([128, 128], dt.bfloat16, tag="ident")
        nc.scalar.dma_start(id_sb[:], ident[:])
        wa_sb = const.tile([128, 128], dt.bfloat16, tag="wa")
        nc.scalar.dma_start(wa_sb[:], wa[:])
        wb_sb = const.tile([128, 128], dt.bfloat16, tag="wb")
        nc.scalar.dma_start(wb_sb[:], wb[:])
        bias_sb = const.tile([128, 1], dt.float32, tag="bias")
        nc.scalar.dma_start(bias_sb[:], biasv[:])
        xt_sb = const.tile([128, MT, BF], dt.bfloat16, tag="xt")
        nc.scalar.dma_start(xt_sb[:], xt[:])

        # PE warmup without any DMA dependency: memset a tile, then dummy
        # matmuls so the HAM clock-gate opens before the first real chunk.
        wz = const.tile([128, 128], dt.bfloat16, tag="wz")
        nc.vector.memset(wz[:], 0.0)
        pwarm = psA.tile([128, 128], dt.float32, tag="ps", name="ps_warm")
        for i in range(36):
            nc.tensor.matmul(
                pwarm[:], wz[:], wz[:], start=(i == 0), stop=(i == 35)
            )

        lxn_tiles = [None] * MT
        ps1_tiles = [None] * MT

        def scatter(w, offp, offx):
            cp = cpw[w]
            pt = ppool.tile([128, cp * 128], dt.bfloat16, tag="pt")
            nc.sync.dma_start(pt[:], pmat[:, offp : offp + cp * 128])
            xgt = xgpool.tile([128, cp * BF], dt.float8e3, tag="xgt")
            nc.sync.dma_start(xgt[:], xg[:, offx : offx + cp * BF])
            ps1 = psA.tile([128, BF], dt.float32, tag="ps", name=f"ps1_{w}")
            for ci in range(cp):
                nc.tensor.matmul(
                    ps1[:],
                    pt[:, ci * 128 : (ci + 1) * 128],
                    xgt[:, ci * BF : (ci + 1) * BF],
                    start=(ci == 0),
                    stop=(ci == cp - 1),
                )
            ps1_tiles[w] = ps1

        lxt_tiles = [None] * MT

        def transp(w):
            # node-major Lx (psum fp32) -> sbuf bf16, then per-batch PE
            # transpose [node, f] -> [f, node]
            lxn = lxnpool.tile([128, BF], dt.bfloat16, tag="lxn")
            nc.vector.tensor_copy(lxn[:], ps1_tiles[w][:])
            ptr = psT.tile([128, BF], dt.bfloat16, tag="pst", name=f"pst_{w}")
            for b in range(B):
                nc.tensor.transpose(
                    ptr[:, b * 128 : (b + 1) * 128],
                    lxn[:, b * 128 : (b + 1) * 128],
                    id_sb[:],
                )
            lxt = lxtpool.tile([128, BF], dt.bfloat16, tag="lxt")
            nc.vector.tensor_copy(lxt[:], ptr[:])
            lxt_tiles[w] = lxt

        def phase2(w):
            # out_T = A^T x^T + B^T Lx^T (+bias), one 128-col slab per batch
            lxt = lxt_tiles[w]
            ps2 = psB.tile([128, BF], dt.float32, tag="ps2", name=f"ps2_{w}")
            for b in range(B):
                sl = slice(b * 128, (b + 1) * 128)
                nc.tensor.matmul(
                    ps2[:, sl], wa_sb[:], xt_sb[:, w, sl], start=True, stop=False
                )
                nc.tensor.matmul(
                    ps2[:, sl], wb_sb[:], lxt[:, sl], start=False, stop=True
                )
            ot = outpool.tile([128, BF], dt.bfloat16, tag="ot")
            nc.scalar.activation(
                ot[:], ps2[:],
                mybir.ActivationFunctionType.Identity,
                bias=bias_sb[:],
            )
            nc.gpsimd.dma_start(out_t[w], ot[:])

        # software-pipelined two windows deep: the PE runs scatter(w),
        # transposes of w-1, feature matmuls of w-2, so every cross-engine
        # handoff (PSUM->SBUF copies on Vector) has a full window of slack
        offp = offx = 0
        for w in range(MT):
            scatter(w, offp, offx)
            offp += cpw[w] * 128
            offx += cpw[w] * BF
            if w > 0:
                transp(w - 1)
            if w > 1:
                phase2(w - 2)
        transp(MT - 1)
        phase2(MT - 2)
        phase2(MT - 1)

    return nc


def _get_nc(cpw):
    key = ("nc", tuple(cpw))
    if key not in _state:
        nc = _build_nc(cpw)
        nc.compile()
        _state[key] = nc
    return _state[key]


def _prep_inputs(x, edge_index, edge_weight, weight, bias):
    """Host-side graph preprocessing -> per-core device input maps."""
    bf16 = ml_dtypes.bfloat16
    row = np.asarray(edge_index[0], dtype=np.int64)
    col = np.asarray(edge_index[1], dtype=np.int64)
    w = np.asarray(edge_weight, dtype=np.float32)

    deg = np.bincount(row, weights=w.astype(np.float64), minlength=N_NODES)
    deg = deg.astype(np.float32)
    dinv = np.where(deg > 0, np.where(deg > 0, deg, 1.0) ** -0.5, 0.0).astype(
        np.float32
    )
    lap2 = (-2.0 * dinv[row] * w * dinv[col]).astype(np.float32)

    # append self loops as ordinary edges
    loops = np.arange(N_NODES, dtype=np.int64)
    rows_all = np.concatenate([row, loops])
    cols_all = np.concatenate([col, loops])
    laps_all = np.concatenate(
        [lap2, np.full(N_NODES, 2.0 * SELF_LOOP_FILL, np.float32)]
    )

    # bucket edges by (core, window) = destination row // 128, derive a
    # shared (SPMD) chunks-per-window schedule covering the fullest core
    g = rows_all // 128                                  # global window id
    cnt = np.bincount(g, minlength=NCORES * MT)
    cpw = np.maximum(
        (cnt.reshape(NCORES, MT).max(axis=0) + 127) // 128, 1
    ).astype(np.int64)
    ct = int(cpw.sum())                                  # chunks per core
    cum = np.zeros(MT, np.int64)
    cum[1:] = np.cumsum(cpw)[:-1]

    order = np.argsort(g, kind="stable")
    gs = g[order]
    starts = np.zeros(NCORES * MT + 1, np.int64)
    starts[1:] = np.cumsum(cnt)
    rank = np.arange(gs.size, dtype=np.int64) - starts[gs]
    cs = gs // MT
    slot = cum[gs % MT] * 128 + rank                     # slot within core
    p = slot % 128                                       # partition (edge lane)
    ci = slot // 128                                     # chunk within core
    j = rows_all[order] % 128                            # dst row within window

    srcs = np.zeros((NCORES, ct * 128), np.int64)
    srcs[cs, slot] = cols_all[order]
    pm = np.zeros((NCORES, 128, ct * 128), np.float32)
    pm[cs, p, ci * 128 + j] = laps_all[order]
    pmat = pm.astype(bf16)
    del pm

    xf = np.asarray(x, np.float32)
    W = np.asarray(weight, dtype=np.float32)
    A = W[0] - W[2]
    Bm = W[1] + 2.0 * W[2] + W[3]
    biasv = np.asarray(bias, dtype=np.float32).reshape(128, 1)
    identity = np.eye(128, dtype=np.float32)

    xn_pad = np.zeros((NPAD, B, F), np.float32)
    xn_pad[:N_NODES] = np.transpose(xf, (1, 0, 2))

    in_maps = []
    for c in range(NCORES):
        S = srcs[c].reshape(ct, 128)
        # xg[e_lane, ci, b*128+f] = x[b, src(ci, e_lane), f]  (fp8 e3m4)
        xgc = np.ascontiguousarray(
            np.transpose(xf[:, S, :], (2, 1, 0, 3)).reshape(128, ct * BF)
        ).astype(ml_dtypes.float8_e3m4)
        r0 = c * MROWS
        # xt[f, w, b*128+j] = x[b, r0 + w*128 + j, f]
        xtc = np.ascontiguousarray(
            xn_pad[r0 : r0 + MROWS]
            .reshape(MT, 128, B, F)
            .transpose(3, 0, 2, 1)
            .reshape(128, MT, BF)
        ).astype(bf16)
        in_maps.append(
            {
                "pmat": np.ascontiguousarray(pmat[c]),
                "xg": xgc,
                "xt": xtc,
                "wa": A.astype(bf16),
                "wb": Bm.astype(bf16),
                "biasv": biasv,
                "ident": identity.astype(bf16),
            }
        )
    return in_maps, tuple(int(v) for v in cpw)


def _ensure_ntff_hook():
    """Register the axon NTFF profiling hook if the image's antenv lacks it.

    The boot path degrades silently when ``antenv.axon_hooks`` is missing;
    recreate the tiny get/set holder and wire it to libaxon_pjrt.so so
    ``run_bass_kernel_spmd(trace=True)`` can capture NTFF profiles.
    """
    import sys
    import types

    try:
        from antenv.axon_hooks import get_axon_ntff_profile_hook  # noqa: F401

        return
    except ImportError:
        pass
    mod = types.ModuleType("antenv.axon_hooks")
    holder = {}
    mod.set_axon_ntff_profile_hook = lambda h: holder.__setitem__("h", h)
    mod.get_axon_ntff_profile_hook = lambda: holder.get("h")
    sys.modules["antenv.axon_hooks"] = mod
    import antenv

    antenv.axon_hooks = mod
    from trn_agent_boot.trn_boot import _ntff_profile_via_ctypes

    hook = _ntff_profile_via_ctypes("/opt/axon/libaxon_pjrt.so")
    if hook is not None:
        mod.set_axon_ntff_profile_hook(hook)


def kernel(x, edge_index, edge_weight, weight, bias):
    import os

    from concourse.bass_utils import run_bass_kernel_spmd

    x = np.asarray(x, dtype=np.float32)
    in_maps, cpw = _prep_inputs(x, edge_index, edge_weight, weight, bias)
    nc = _get_nc(cpw)
    trace = bool(int(os.environ.get("CHEB_TRACE", "0")))
    if trace:
        _ensure_ntff_hook()
    res = run_bass_kernel_spmd(nc, in_maps, list(range(NCORES)), trace=trace)
    _state["last_result"] = res
    # out_t[w, f, b*128+j] (per core) -> out[b, c*MROWS + w*128 + j, f]
    parts = []
    for c in range(NCORES):
        r = np.asarray(res.results[c]["out_t"], dtype=np.float32)
        parts.append(
            r.reshape(MT, 128, B, 128).transpose(2, 0, 3, 1).reshape(B, MROWS, F)
        )
    out = np.concatenate(parts, axis=1)[:, :N_NODES, :]
    return np.ascontiguousarray(out)
